# revision 21
# baseline (speedup 1.0000x reference)
"""Trainium2 Bass kernel for C3ALinear: y = x @ W.T + b + block_circconv(x, k)/D.

The block-circular convolution is algebraically a matmul with a block-circulant
matrix, so the whole op folds into a single matmul with
W_comb = base_weight + circulant_expand(c3a_kernel)/D_IN.  The 8192x4096x4096
matmul runs on 8 NeuronCores with host-cast bf16 operands (rel err ~2.3e-3,
comfortably under the 2e-2 gate).

Production layout "wstat8c": 8-way token sharding (x/core = 8 MB bf16,
SBUF-resident), full D_OUT per core with the packed W (32 MB bf16, replicated)
streamed as per-d-tile slabs over the whole kernel.  W-stationary 128x128
tiles against 512-token moving blocks; head phase pairs d0/d1 k-outer so the
x stream is fully hidden (4 MMs/k ~1.1us vs ~0.8us/k DMA); steady state is
t-outer/k-inner with immediate per-bank PSUM eviction; the first W slab is
chunked (64 KB gate) and the last d-tile tapers its sweep widths
(512/256/128/128) to shrink the startup and drain exposure.  Measured on sustained
8-core load the PE streams moving columns at ~1.95 GHz effective (P0
downclock from 2.4), making the kernel PE-bound within ~1-2% of the pure
matmul-stream floor; instruction count / LDWEIGHTS / semaphore traffic were
measured to be fully hidden (N=512/256/128 ablations all time identical).
"""
import sys

sys.path.insert(0, "/opt/trn_rl_repo")

import numpy as np

B, S, D_IN, D_OUT, BLK = 4, 2048, 4096, 4096, 256
N_CORES = 8
TOK = B * S              # 8192 tokens
TOK_SH = TOK // N_CORES  # 1024 tokens per core
P = 128                  # partitions
NF = 512                 # matmul free dim (one PSUM bank of fp32)
KT = D_IN // P           # 32 contraction tiles
MT = TOK_SH // P         # 8 token tiles per core
NT = D_OUT // NF         # 8 output-feature panels

_cache = {}


def _build_nc(repeats=None, evict="any", wr_bufs=10, ws_bufs=6, out_ring="sync"):
    import contextlib

    import concourse.mybir as mybir
    import concourse.tile as tile
    from concourse import bacc

    nc = bacc.Bacc(None, target_bir_lowering=False, debug=False)

    xT = nc.dram_tensor("xT", [D_IN, TOK_SH], mybir.dt.float32, kind="ExternalInput")
    wT = nc.dram_tensor("wT", [D_IN, D_OUT], mybir.dt.float32, kind="ExternalInput")
    biasb = nc.dram_tensor("biasb", [P, D_OUT], mybir.dt.float32, kind="ExternalInput")
    out = nc.dram_tensor("out", [TOK_SH, D_OUT], mybir.dt.float32, kind="ExternalOutput")

    evict_eng = {"any": nc.any, "vector": nc.vector}[evict]
    out_eng = {"sync": nc.sync, "scalar": nc.scalar}[out_ring]

    with tile.TileContext(nc) as tc:
        with tc.tile_pool(name="xs", bufs=2) as xs_pool, \
             tc.tile_pool(name="ws", bufs=ws_bufs) as ws_pool, \
             tc.tile_pool(name="wr", bufs=wr_bufs) as wr_pool, \
             tc.tile_pool(name="bias", bufs=2) as bias_pool, \
             tc.tile_pool(name="ob", bufs=6) as ob_pool, \
             tc.tile_pool(name="xr", bufs=KT) as xr_pool, \
             tc.tile_pool(name="ps", bufs=8, space="PSUM") as ps_pool:

            if repeats is not None:
                loop_cm = tc.For_i(
                    0, repeats, 1,
                    hint_engines=(
                        mybir.EngineType.PE, mybir.EngineType.DVE,
                        mybir.EngineType.Activation, mybir.EngineType.SP,
                        mybir.EngineType.Pool,
                    ),
                )
            else:
                loop_cm = contextlib.nullcontext()

            with loop_cm:
                # x shard loads are interleaved into panel 0's k-loop so the
                # first W tile isn't queued behind 16 MB of x DMA.
                x_r = [None] * KT

                for n in range(NT):
                    bias_t = bias_pool.tile([P, NF], mybir.dt.float32, tag="bias")
                    nc.sync.dma_start(bias_t[:], biasb[:, n * NF:(n + 1) * NF])
                    psums = [
                        ps_pool.tile([P, NF], mybir.dt.float32, tag="ps", name="ps")
                        for _ in range(MT)
                    ]
                    for k in range(KT):
                        if n == 0:
                            xs = xs_pool.tile([P, TOK_SH], mybir.dt.float32, tag="xs")
                            nc.sync.dma_start(xs[:], xT[k * P:(k + 1) * P, :])
                            xr = xr_pool.tile([P, TOK_SH], mybir.dt.float32r, tag="xr")
                            nc.vector.tensor_copy(xr[:], xs[:])
                            x_r[k] = xr
                        ws = ws_pool.tile([P, NF], mybir.dt.float32, tag="ws")
                        nc.sync.dma_start(
                            ws[:], wT[k * P:(k + 1) * P, n * NF:(n + 1) * NF])
                        wr = wr_pool.tile([P, NF], mybir.dt.float32r, tag="wr")
                        nc.vector.tensor_copy(wr[:], ws[:])
                        for m in range(MT):
                            nc.tensor.matmul(
                                psums[m][:],
                                x_r[k][:, m * P:(m + 1) * P],
                                wr[:],
                                start=(k == 0),
                                stop=(k == KT - 1),
                            )
                    for m in range(MT):
                        ob = ob_pool.tile([P, NF], mybir.dt.float32, tag="ob")
                        evict_eng.tensor_add(ob[:], psums[m][:], bias_t[:])
                        out_eng.dma_start(
                            out[m * P:(m + 1) * P, n * NF:(n + 1) * NF], ob[:]
                        )

    nc.compile()
    return nc


def _build_nc_kb(repeats=None, KB=8):
    """xstat with kb-blocked inner loop: 8 W k-tiles held resident per block,
    m-loop outside k so each PSUM bank receives 8 consecutive MMs (8x fewer
    bank switches than bank-per-MM; see K18 psum-cycling micro-idle)."""
    import contextlib

    import concourse.mybir as mybir
    import concourse.tile as tile
    from concourse import bacc

    NKB = KT // KB  # 4 blocks of 8 k-tiles

    nc = bacc.Bacc(None, target_bir_lowering=False, debug=False)

    xT = nc.dram_tensor("xT", [D_IN, TOK_SH], mybir.dt.float32, kind="ExternalInput")
    wT = nc.dram_tensor("wT", [D_IN, D_OUT], mybir.dt.float32, kind="ExternalInput")
    biasb = nc.dram_tensor("biasb", [P, D_OUT], mybir.dt.float32, kind="ExternalInput")
    out = nc.dram_tensor("out", [TOK_SH, D_OUT], mybir.dt.float32, kind="ExternalOutput")

    with tile.TileContext(nc) as tc:
        with tc.tile_pool(name="xs", bufs=2) as xs_pool, \
             tc.tile_pool(name="ws", bufs=4) as ws_pool, \
             tc.tile_pool(name="wr", bufs=2 * KB) as wr_pool, \
             tc.tile_pool(name="bias", bufs=2) as bias_pool, \
             tc.tile_pool(name="ob", bufs=4) as ob_pool, \
             tc.tile_pool(name="xr", bufs=KT) as xr_pool, \
             tc.tile_pool(name="ps", bufs=8, space="PSUM") as ps_pool:

            if repeats is not None:
                loop_cm = tc.For_i(
                    0, repeats, 1,
                    hint_engines=(
                        mybir.EngineType.PE, mybir.EngineType.DVE,
                        mybir.EngineType.Activation, mybir.EngineType.SP,
                        mybir.EngineType.Pool,
                    ),
                )
            else:
                loop_cm = contextlib.nullcontext()

            with loop_cm:
                x_r = [None] * KT

                for n in range(NT):
                    bias_t = bias_pool.tile([P, NF], mybir.dt.float32, tag="bias")
                    nc.sync.dma_start(bias_t[:], biasb[:, n * NF:(n + 1) * NF])
                    psums = [
                        ps_pool.tile([P, NF], mybir.dt.float32, tag="ps", name="ps")
                        for _ in range(MT)
                    ]
                    for kb in range(NKB):
                        wtiles = []
                        for kk in range(KB):
                            k = kb * KB + kk
                            if n == 0:
                                xs = xs_pool.tile(
                                    [P, TOK_SH], mybir.dt.float32, tag="xs")
                                nc.sync.dma_start(xs[:], xT[k * P:(k + 1) * P, :])
                                xr = xr_pool.tile(
                                    [P, TOK_SH], mybir.dt.float32r, tag="xr")
                                nc.vector.tensor_copy(xr[:], xs[:])
                                x_r[k] = xr
                            ws = ws_pool.tile([P, NF], mybir.dt.float32, tag="ws")
                            nc.sync.dma_start(
                                ws[:], wT[k * P:(k + 1) * P, n * NF:(n + 1) * NF])
                            wr = wr_pool.tile([P, NF], mybir.dt.float32r, tag="wr")
                            nc.vector.tensor_copy(wr[:], ws[:])
                            wtiles.append(wr)
                        for m in range(MT):
                            for kk in range(KB):
                                k = kb * KB + kk
                                nc.tensor.matmul(
                                    psums[m][:],
                                    x_r[k][:, m * P:(m + 1) * P],
                                    wtiles[kk][:],
                                    start=(k == 0),
                                    stop=(k == KT - 1),
                                )
                    for m in range(MT):
                        ob = ob_pool.tile([P, NF], mybir.dt.float32, tag="ob")
                        nc.any.tensor_add(ob[:], psums[m][:], bias_t[:])
                        nc.sync.dma_start(
                            out[m * P:(m + 1) * P, n * NF:(n + 1) * NF], ob[:]
                        )

    nc.compile()
    return nc


def _build_nc_v2(repeats=None):
    """xstat with in-place f32r rounding (no f32 staging pools: DMA lands in
    the f32r tile via a bitcast view, DVE rounds in place), W DMAs on the
    sync HWDGE ring, x/out DMAs on the scalar HWDGE ring, wider wr pool."""
    import contextlib

    import concourse.mybir as mybir
    import concourse.tile as tile
    from concourse import bacc

    nc = bacc.Bacc(None, target_bir_lowering=False, debug=False)

    f32 = mybir.dt.float32
    f32r = mybir.dt.float32r
    xT = nc.dram_tensor("xT", [D_IN, TOK_SH], f32, kind="ExternalInput")
    wT = nc.dram_tensor("wT", [D_IN, D_OUT], f32, kind="ExternalInput")
    biasb = nc.dram_tensor("biasb", [P, D_OUT], f32, kind="ExternalInput")
    out = nc.dram_tensor("out", [TOK_SH, D_OUT], f32, kind="ExternalOutput")

    with tile.TileContext(nc) as tc:
        with tc.tile_pool(name="wr", bufs=16) as wr_pool, \
             tc.tile_pool(name="bias", bufs=2) as bias_pool, \
             tc.tile_pool(name="ob", bufs=8) as ob_pool, \
             tc.tile_pool(name="xr", bufs=KT) as xr_pool, \
             tc.tile_pool(name="ps", bufs=8, space="PSUM") as ps_pool:

            if repeats is not None:
                loop_cm = tc.For_i(
                    0, repeats, 1,
                    hint_engines=(
                        mybir.EngineType.PE, mybir.EngineType.DVE,
                        mybir.EngineType.Activation, mybir.EngineType.SP,
                        mybir.EngineType.Pool,
                    ),
                )
            else:
                loop_cm = contextlib.nullcontext()

            with loop_cm:
                x_r = [None] * KT

                for n in range(NT):
                    bias_t = bias_pool.tile([P, NF], f32, tag="bias")
                    nc.sync.dma_start(bias_t[:], biasb[:, n * NF:(n + 1) * NF])
                    psums = [
                        ps_pool.tile([P, NF], f32, tag="ps", name="ps")
                        for _ in range(MT)
                    ]
                    for k in range(KT):
                        if n == 0:
                            xr = xr_pool.tile([P, TOK_SH], f32r, tag="xr")
                            nc.scalar.dma_start(
                                xr[:].bitcast(f32), xT[k * P:(k + 1) * P, :])
                            nc.vector.tensor_copy(xr[:], xr[:].bitcast(f32))
                            x_r[k] = xr
                        wr = wr_pool.tile([P, NF], f32r, tag="wr")
                        nc.sync.dma_start(
                            wr[:].bitcast(f32),
                            wT[k * P:(k + 1) * P, n * NF:(n + 1) * NF])
                        nc.vector.tensor_copy(wr[:], wr[:].bitcast(f32))
                        for m in range(MT):
                            nc.tensor.matmul(
                                psums[m][:],
                                x_r[k][:, m * P:(m + 1) * P],
                                wr[:],
                                start=(k == 0),
                                stop=(k == KT - 1),
                            )
                    for m in range(MT):
                        ob = ob_pool.tile([P, NF], f32, tag="ob")
                        nc.any.tensor_add(ob[:], psums[m][:], bias_t[:])
                        nc.scalar.dma_start(
                            out[m * P:(m + 1) * P, n * NF:(n + 1) * NF], ob[:]
                        )

    nc.compile()
    return nc


def _build_nc_wstat(repeats=None):
    """W-stationary layout: out.T[d_out, tok] per core; lhsT = W tile reused
    across 2 moving token-blocks (halves exposed f32r weight-load cost);
    bias is per-partition via tensor_scalar_add."""
    import contextlib

    import concourse.mybir as mybir
    import concourse.tile as tile
    from concourse import bacc

    DG = 8            # d_out groups of NF=512 (4 d-tiles of 128)
    TB = TOK_SH // NF  # 2 token blocks

    nc = bacc.Bacc(None, target_bir_lowering=False, debug=False)

    xT = nc.dram_tensor("xT", [D_IN, TOK_SH], mybir.dt.float32, kind="ExternalInput")
    wT = nc.dram_tensor("wT", [D_IN, D_OUT], mybir.dt.float32, kind="ExternalInput")
    bias_col = nc.dram_tensor(
        "bias_col", [P, D_OUT // P], mybir.dt.float32, kind="ExternalInput")
    outT = nc.dram_tensor(
        "outT", [D_OUT, TOK_SH], mybir.dt.float32, kind="ExternalOutput")

    with tile.TileContext(nc) as tc:
        with tc.tile_pool(name="xs", bufs=2) as xs_pool, \
             tc.tile_pool(name="xr", bufs=KT) as xr_pool, \
             tc.tile_pool(name="ws", bufs=4) as ws_pool, \
             tc.tile_pool(name="wr", bufs=6) as wr_pool, \
             tc.tile_pool(name="bias", bufs=1) as bias_pool, \
             tc.tile_pool(name="ob", bufs=4) as ob_pool, \
             tc.tile_pool(name="ps", bufs=8, space="PSUM") as ps_pool:

            if repeats is not None:
                loop_cm = tc.For_i(
                    0, repeats, 1,
                    hint_engines=(
                        mybir.EngineType.PE, mybir.EngineType.DVE,
                        mybir.EngineType.Activation, mybir.EngineType.SP,
                        mybir.EngineType.Pool,
                    ),
                )
            else:
                loop_cm = contextlib.nullcontext()

            with loop_cm:
                bias_t = bias_pool.tile([P, D_OUT // P], mybir.dt.float32, tag="bias")
                nc.sync.dma_start(bias_t[:], bias_col[:])

                x_r = [None] * KT

                for dg in range(DG):
                    psums = [
                        ps_pool.tile([P, NF], mybir.dt.float32, tag="ps", name="ps")
                        for _ in range(4 * TB)
                    ]
                    for k in range(KT):
                        if dg == 0:
                            xs = xs_pool.tile([P, TOK_SH], mybir.dt.float32, tag="xs")
                            nc.sync.dma_start(xs[:], xT[k * P:(k + 1) * P, :])
                            xr = xr_pool.tile([P, TOK_SH], mybir.dt.float32r, tag="xr")
                            nc.vector.tensor_copy(xr[:], xs[:])
                            x_r[k] = xr
                        ws = ws_pool.tile([P, NF], mybir.dt.float32, tag="ws")
                        nc.sync.dma_start(
                            ws[:], wT[k * P:(k + 1) * P, dg * NF:(dg + 1) * NF])
                        wr = wr_pool.tile([P, NF], mybir.dt.float32r, tag="wr")
                        nc.vector.tensor_copy(wr[:], ws[:])
                        for j in range(4):
                            for t in range(TB):
                                nc.tensor.matmul(
                                    psums[j * TB + t][:],
                                    wr[:, j * P:(j + 1) * P],
                                    x_r[k][:, t * NF:(t + 1) * NF],
                                    start=(k == 0),
                                    stop=(k == KT - 1),
                                )
                    for j in range(4):
                        d = dg * 4 + j
                        for t in range(TB):
                            ob = ob_pool.tile([P, NF], mybir.dt.float32, tag="ob")
                            nc.vector.tensor_scalar_add(
                                ob[:], psums[j * TB + t][:], bias_t[:, d:d + 1])
                            nc.sync.dma_start(
                                outT[d * P:(d + 1) * P, t * NF:(t + 1) * NF], ob[:])

    nc.compile()
    return nc


def _build_nc_wstat2(repeats=None):
    """W-stationary with half-group PSUM alternation: each d-group of 512
    outputs is processed as two halves of 4 PSUM tiles alternating between
    bank groups 0-3 and 4-7, so evictions of one half overlap compute of the
    next and the PE never waits on PSUM recycling.  W is read once as
    [128,256] half-tiles; x stays resident in f32r."""
    import contextlib

    import concourse.mybir as mybir
    import concourse.tile as tile
    from concourse import bacc

    DG = 8             # d_out groups of NF=512
    TB = TOK_SH // NF  # 2 token blocks
    HNF = NF // 2      # 256: W half-tile width

    nc = bacc.Bacc(None, target_bir_lowering=False, debug=False)

    xT = nc.dram_tensor("xT", [D_IN, TOK_SH], mybir.dt.float32, kind="ExternalInput")
    wT = nc.dram_tensor("wT", [D_IN, D_OUT], mybir.dt.float32, kind="ExternalInput")
    bias_col = nc.dram_tensor(
        "bias_col", [P, D_OUT // P], mybir.dt.float32, kind="ExternalInput")
    outT = nc.dram_tensor(
        "outT", [D_OUT, TOK_SH], mybir.dt.float32, kind="ExternalOutput")

    with tile.TileContext(nc) as tc:
        with tc.tile_pool(name="xs", bufs=2) as xs_pool, \
             tc.tile_pool(name="xr", bufs=KT) as xr_pool, \
             tc.tile_pool(name="ws", bufs=6) as ws_pool, \
             tc.tile_pool(name="wr", bufs=8) as wr_pool, \
             tc.tile_pool(name="bias", bufs=1) as bias_pool, \
             tc.tile_pool(name="ob", bufs=6) as ob_pool, \
             tc.tile_pool(name="ps", bufs=8, space="PSUM") as ps_pool:

            if repeats is not None:
                loop_cm = tc.For_i(
                    0, repeats, 1,
                    hint_engines=(
                        mybir.EngineType.PE, mybir.EngineType.DVE,
                        mybir.EngineType.Activation, mybir.EngineType.SP,
                        mybir.EngineType.Pool,
                    ),
                )
            else:
                loop_cm = contextlib.nullcontext()

            with loop_cm:
                bias_t = bias_pool.tile([P, D_OUT // P], mybir.dt.float32, tag="bias")
                nc.sync.dma_start(bias_t[:], bias_col[:])

                x_r = [None] * KT

                def evict(dg, h, psums):
                    # evictions of half (dg,h): d tiles dg*4+2h, dg*4+2h+1
                    evs = []
                    for j in range(2):
                        d = dg * 4 + 2 * h + j
                        for t in range(TB):
                            evs.append((d, t, psums[j * TB + t]))
                    return evs

                def emit_evict(ev):
                    d, t, psum = ev
                    ob = ob_pool.tile([P, NF], mybir.dt.float32, tag="ob", name="ob")
                    nc.vector.tensor_scalar_add(ob[:], psum[:], bias_t[:, d:d + 1])
                    nc.sync.dma_start(
                        outT[d * P:(d + 1) * P, t * NF:(t + 1) * NF], ob[:])

                pending = []
                for dg in range(DG):
                    for h in range(2):
                        psums = [
                            ps_pool.tile([P, NF], mybir.dt.float32, tag="ps",
                                         name="ps")
                            for _ in range(4)
                        ]
                        for k in range(KT):
                            if dg == 0 and h == 0:
                                xs = xs_pool.tile(
                                    [P, TOK_SH], mybir.dt.float32, tag="xs")
                                nc.sync.dma_start(xs[:], xT[k * P:(k + 1) * P, :])
                                xr = xr_pool.tile(
                                    [P, TOK_SH], mybir.dt.float32r, tag="xr")
                                nc.vector.tensor_copy(xr[:], xs[:])
                                x_r[k] = xr
                            ws = ws_pool.tile([P, HNF], mybir.dt.float32, tag="ws")
                            nc.sync.dma_start(
                                ws[:],
                                wT[k * P:(k + 1) * P,
                                   dg * NF + h * HNF:dg * NF + (h + 1) * HNF])
                            wr = wr_pool.tile([P, HNF], mybir.dt.float32r, tag="wr")
                            nc.vector.tensor_copy(wr[:], ws[:])
                            if pending and k < len(pending):
                                emit_evict(pending[k])
                            for j in range(2):
                                for t in range(TB):
                                    nc.tensor.matmul(
                                        psums[j * TB + t][:],
                                        wr[:, j * P:(j + 1) * P],
                                        x_r[k][:, t * NF:(t + 1) * NF],
                                        start=(k == 0),
                                        stop=(k == KT - 1),
                                    )
                        pending = evict(dg, h, psums)
                for ev in pending:
                    emit_evict(ev)

    nc.compile()
    return nc


def _build_nc_bf16(repeats=None, wr_bufs=16, ob_bufs=8):
    """xstat with host-cast bf16 operands: x and W arrive as bf16 so there are
    no on-device dtype conversions; bf16 weight loads get FWL + background
    weight-buffer pull-ahead, unlike fp32r's serial in-matmul 4-byte load."""
    import contextlib

    import concourse.mybir as mybir
    import concourse.tile as tile
    from concourse import bacc

    nc = bacc.Bacc(None, target_bir_lowering=False, debug=False)

    f32 = mybir.dt.float32
    bf16 = mybir.dt.bfloat16
    xT = nc.dram_tensor("xT", [D_IN, TOK_SH], bf16, kind="ExternalInput")
    wT = nc.dram_tensor("wT", [D_IN, D_OUT], bf16, kind="ExternalInput")
    biasb = nc.dram_tensor("biasb", [P, D_OUT], f32, kind="ExternalInput")
    out = nc.dram_tensor("out", [TOK_SH, D_OUT], f32, kind="ExternalOutput")

    with tile.TileContext(nc) as tc:
        with tc.tile_pool(name="wr", bufs=wr_bufs) as wr_pool, \
             tc.tile_pool(name="bias", bufs=2) as bias_pool, \
             tc.tile_pool(name="ob", bufs=ob_bufs) as ob_pool, \
             tc.tile_pool(name="xr", bufs=KT) as xr_pool, \
             tc.tile_pool(name="ps", bufs=8, space="PSUM") as ps_pool:

            if repeats is not None:
                loop_cm = tc.For_i(
                    0, repeats, 1,
                    hint_engines=(
                        mybir.EngineType.PE, mybir.EngineType.DVE,
                        mybir.EngineType.Activation, mybir.EngineType.SP,
                        mybir.EngineType.Pool,
                    ),
                )
            else:
                loop_cm = contextlib.nullcontext()

            with loop_cm:
                x_r = [None] * KT

                for n in range(NT):
                    bias_t = bias_pool.tile([P, NF], f32, tag="bias")
                    nc.sync.dma_start(bias_t[:], biasb[:, n * NF:(n + 1) * NF])
                    psums = [
                        ps_pool.tile([P, NF], f32, tag="ps", name="ps")
                        for _ in range(MT)
                    ]
                    for k in range(KT):
                        if n == 0:
                            xr = xr_pool.tile([P, TOK_SH], bf16, tag="xr")
                            nc.scalar.dma_start(xr[:], xT[k * P:(k + 1) * P, :])
                            x_r[k] = xr
                        wr = wr_pool.tile([P, NF], bf16, tag="wr")
                        nc.sync.dma_start(
                            wr[:], wT[k * P:(k + 1) * P, n * NF:(n + 1) * NF])
                        for m in range(MT):
                            nc.tensor.matmul(
                                psums[m][:],
                                x_r[k][:, m * P:(m + 1) * P],
                                wr[:],
                                start=(k == 0),
                                stop=(k == KT - 1),
                            )
                    for m in range(MT):
                        ob = ob_pool.tile([P, NF], f32, tag="ob")
                        nc.any.tensor_add(ob[:], psums[m][:], bias_t[:])
                        nc.scalar.dma_start(
                            out[m * P:(m + 1) * P, n * NF:(n + 1) * NF], ob[:]
                        )

    nc.compile()
    return nc


def _build_nc_wstat_bf16(repeats=None):
    """W-stationary bf16: out.T[d_out, tok]; stationary = 128-col W slice
    reused across TB=2 moving 512-token blocks (halves LDWEIGHTS), x resident
    bf16, no on-device dtype conversions."""
    import contextlib

    import concourse.mybir as mybir
    import concourse.tile as tile
    from concourse import bacc

    DG = 8            # d_out groups of NF=512 (4 d-tiles of 128)
    TB = TOK_SH // NF  # 2 token blocks

    nc = bacc.Bacc(None, target_bir_lowering=False, debug=False)

    f32 = mybir.dt.float32
    bf16 = mybir.dt.bfloat16
    xT = nc.dram_tensor("xT", [D_IN, TOK_SH], bf16, kind="ExternalInput")
    wT = nc.dram_tensor("wT", [D_IN, D_OUT], bf16, kind="ExternalInput")
    bias_col = nc.dram_tensor(
        "bias_col", [P, D_OUT // P], f32, kind="ExternalInput")
    outT = nc.dram_tensor(
        "outT", [D_OUT, TOK_SH], f32, kind="ExternalOutput")

    with tile.TileContext(nc) as tc:
        with tc.tile_pool(name="xr", bufs=KT) as xr_pool, \
             tc.tile_pool(name="wr", bufs=8) as wr_pool, \
             tc.tile_pool(name="bias", bufs=1) as bias_pool, \
             tc.tile_pool(name="ob", bufs=8) as ob_pool, \
             tc.tile_pool(name="ps", bufs=8, space="PSUM") as ps_pool:

            if repeats is not None:
                loop_cm = tc.For_i(
                    0, repeats, 1,
                    hint_engines=(
                        mybir.EngineType.PE, mybir.EngineType.DVE,
                        mybir.EngineType.Activation, mybir.EngineType.SP,
                        mybir.EngineType.Pool,
                    ),
                )
            else:
                loop_cm = contextlib.nullcontext()

            with loop_cm:
                bias_t = bias_pool.tile([P, D_OUT // P], f32, tag="bias")
                nc.sync.dma_start(bias_t[:], bias_col[:])

                x_r = [None] * KT

                for dg in range(DG):
                    psums = [
                        ps_pool.tile([P, NF], f32, tag="ps", name="ps")
                        for _ in range(4 * TB)
                    ]
                    for k in range(KT):
                        if dg == 0:
                            xr = xr_pool.tile([P, TOK_SH], bf16, tag="xr")
                            nc.scalar.dma_start(xr[:], xT[k * P:(k + 1) * P, :])
                            x_r[k] = xr
                        wr = wr_pool.tile([P, NF], bf16, tag="wr")
                        nc.sync.dma_start(
                            wr[:], wT[k * P:(k + 1) * P, dg * NF:(dg + 1) * NF])
                        for j in range(4):
                            for t in range(TB):
                                nc.tensor.matmul(
                                    psums[j * TB + t][:],
                                    wr[:, j * P:(j + 1) * P],
                                    x_r[k][:, t * NF:(t + 1) * NF],
                                    start=(k == 0),
                                    stop=(k == KT - 1),
                                )
                    for j in range(4):
                        d = dg * 4 + j
                        for t in range(TB):
                            ob = ob_pool.tile([P, NF], f32, tag="ob")
                            nc.any.tensor_scalar_add(
                                ob[:], psums[j * TB + t][:], bias_t[:, d:d + 1])
                            nc.scalar.dma_start(
                                outT[d * P:(d + 1) * P, t * NF:(t + 1) * NF], ob[:])

    nc.compile()
    return nc


def _build_nc_bf16v2(repeats=None, wr_bufs=40, ob_bufs=8):
    """xstat bf16 with half-panel PSUM alternation: each 512-wide output panel
    is computed as two halves of 4 m-tiles on alternating PSUM bank groups, so
    evictions of one half overlap the next half's 27us of matmuls and the PE
    never waits on a bank.  The panel's 32 W k-tiles stay resident in SBUF
    across both halves (4 MB), so W is still DMA'd exactly once."""
    import contextlib

    import concourse.mybir as mybir
    import concourse.tile as tile
    from concourse import bacc

    HM = MT // 2  # 4 m-tiles per half

    nc = bacc.Bacc(None, target_bir_lowering=False, debug=False)

    f32 = mybir.dt.float32
    bf16 = mybir.dt.bfloat16
    xT = nc.dram_tensor("xT", [D_IN, TOK_SH], bf16, kind="ExternalInput")
    wT = nc.dram_tensor("wT", [D_IN, D_OUT], bf16, kind="ExternalInput")
    biasb = nc.dram_tensor("biasb", [P, D_OUT], f32, kind="ExternalInput")
    out = nc.dram_tensor("out", [TOK_SH, D_OUT], f32, kind="ExternalOutput")

    with tile.TileContext(nc) as tc:
        with tc.tile_pool(name="wr", bufs=wr_bufs) as wr_pool, \
             tc.tile_pool(name="bias", bufs=2) as bias_pool, \
             tc.tile_pool(name="ob", bufs=ob_bufs) as ob_pool, \
             tc.tile_pool(name="xr", bufs=KT) as xr_pool, \
             tc.tile_pool(name="ps", bufs=8, space="PSUM") as ps_pool:

            if repeats is not None:
                loop_cm = tc.For_i(
                    0, repeats, 1,
                    hint_engines=(
                        mybir.EngineType.PE, mybir.EngineType.DVE,
                        mybir.EngineType.Activation, mybir.EngineType.SP,
                        mybir.EngineType.Pool,
                    ),
                )
            else:
                loop_cm = contextlib.nullcontext()

            with loop_cm:
                x_r = [None] * KT
                evict_engs = [nc.vector, nc.vector]

                for n in range(NT):
                    bias_t = bias_pool.tile([P, NF], f32, tag="bias")
                    nc.sync.dma_start(bias_t[:], biasb[:, n * NF:(n + 1) * NF])
                    wtiles = [None] * KT
                    for h in range(2):
                        psums = [
                            ps_pool.tile([P, NF], f32, tag="ps", name="ps")
                            for _ in range(HM)
                        ]
                        for k in range(KT):
                            if n == 0 and h == 0:
                                xr = xr_pool.tile([P, TOK_SH], bf16, tag="xr")
                                nc.scalar.dma_start(
                                    xr[:], xT[k * P:(k + 1) * P, :])
                                x_r[k] = xr
                            if h == 0:
                                wr = wr_pool.tile([P, NF], bf16, tag="wr")
                                nc.sync.dma_start(
                                    wr[:],
                                    wT[k * P:(k + 1) * P, n * NF:(n + 1) * NF])
                                wtiles[k] = wr
                            for mm in range(HM):
                                m = h * HM + mm
                                nc.tensor.matmul(
                                    psums[mm][:],
                                    x_r[k][:, m * P:(m + 1) * P],
                                    wtiles[k][:],
                                    start=(k == 0),
                                    stop=(k == KT - 1),
                                )
                        for mm in range(HM):
                            m = h * HM + mm
                            ob = ob_pool.tile([P, NF], f32, tag="ob")
                            evict_engs[mm % 2].tensor_add(
                                ob[:], psums[mm][:], bias_t[:])
                            nc.scalar.dma_start(
                                out[m * P:(m + 1) * P, n * NF:(n + 1) * NF],
                                ob[:]
                            )

    nc.compile()
    return nc


D_SH4 = D_OUT // 2    # 2048: dout shard (2-way)
TOK_SH4 = TOK // 4    # 2048: token shard (4-way)
DT4 = D_SH4 // P      # 16 d-tiles per core
TB4 = TOK_SH4 // NF   # 4 moving token blocks per stationary load


def _build_nc_wstat4(repeats=None, wr_bufs=16, ob_bufs=8):
    """W-stationary bf16 with reuse x4: hybrid 4-way-token x 2-way-dout
    sharding gives each core 2048 tokens (resident bf16, 16 MB) and 2048 out
    features.  Each 128x128 W stationary tile is loaded once and streamed
    against 4 moving 512-token blocks, quartering LDWEIGHTS; PSUM naturally
    alternates bank groups 0-3/4-7 between d-tiles so evictions overlap."""
    import contextlib

    import concourse.mybir as mybir
    import concourse.tile as tile
    from concourse import bacc

    nc = bacc.Bacc(None, target_bir_lowering=False, debug=False)

    f32 = mybir.dt.float32
    bf16 = mybir.dt.bfloat16
    xT = nc.dram_tensor("xT", [D_IN, TOK_SH4], bf16, kind="ExternalInput")
    wTs = nc.dram_tensor("wTs", [D_IN, D_SH4], bf16, kind="ExternalInput")
    bias_col = nc.dram_tensor("bias_col", [P, DT4], f32, kind="ExternalInput")
    outT = nc.dram_tensor(
        "outT", [D_SH4, TOK_SH4], f32, kind="ExternalOutput")

    with tile.TileContext(nc) as tc:
        with tc.tile_pool(name="xr", bufs=KT) as xr_pool, \
             tc.tile_pool(name="wr", bufs=wr_bufs) as wr_pool, \
             tc.tile_pool(name="bias", bufs=1) as bias_pool, \
             tc.tile_pool(name="ob", bufs=ob_bufs) as ob_pool, \
             tc.tile_pool(name="ps", bufs=8, space="PSUM") as ps_pool:

            if repeats is not None:
                loop_cm = tc.For_i(
                    0, repeats, 1,
                    hint_engines=(
                        mybir.EngineType.PE, mybir.EngineType.DVE,
                        mybir.EngineType.Activation, mybir.EngineType.SP,
                        mybir.EngineType.Pool,
                    ),
                )
            else:
                loop_cm = contextlib.nullcontext()

            with loop_cm:
                bias_t = bias_pool.tile([P, DT4], f32, tag="bias")
                nc.sync.dma_start(bias_t[:], bias_col[:])
                x_r = [None] * KT

                for d in range(DT4):
                    psums = [
                        ps_pool.tile([P, NF], f32, tag="ps", name="ps")
                        for _ in range(TB4)
                    ]
                    for k in range(KT):
                        if d == 0:
                            xr = xr_pool.tile([P, TOK_SH4], bf16, tag="xr")
                            # split x tile across both DGE rings so the d=0
                            # pass isn't DMA-bound on one ring
                            nc.scalar.dma_start(
                                xr[:, :TOK_SH4 // 2],
                                xT[k * P:(k + 1) * P, :TOK_SH4 // 2])
                            nc.sync.dma_start(
                                xr[:, TOK_SH4 // 2:],
                                xT[k * P:(k + 1) * P, TOK_SH4 // 2:])
                            x_r[k] = xr
                        wr = wr_pool.tile([P, P], bf16, tag="wr")
                        nc.sync.dma_start(
                            wr[:], wTs[k * P:(k + 1) * P, d * P:(d + 1) * P])
                        for t in range(TB4):
                            nc.tensor.matmul(
                                psums[t][:],
                                wr[:],
                                x_r[k][:, t * NF:(t + 1) * NF],
                                start=(k == 0),
                                stop=(k == KT - 1),
                            )
                    for t in range(TB4):
                        ob = ob_pool.tile([P, NF], f32, tag="ob")
                        nc.vector.tensor_scalar_add(
                            ob[:], psums[t][:], bias_t[:, d:d + 1])
                        nc.scalar.dma_start(
                            outT[d * P:(d + 1) * P, t * NF:(t + 1) * NF],
                            ob[:])

    nc.compile()
    return nc


def _build_nc_wstat4b(repeats=None, wr_bufs=32, ob_bufs=8):
    return _build_nc_wstat4b_impl(repeats, wr_bufs, ob_bufs, out_bf16=False)


def _build_nc_wstat4b_impl(repeats, wr_bufs, ob_bufs, out_bf16):
    """wstat4 with a paired d0/d1 start phase: while the 16 MB x shard
    streams in (HBM-bound, ~45us), two d-tiles' worth of matmuls interleave
    per k so the x-arrival wait is amortized over 2 tiles of compute; from
    d=2 on, single d-tiles ping-pong PSUM bank groups so evictions overlap.
    x is striped over 3 DGE rings (scalar/sync/gpsimd), W tiles ride the
    vector ring, outputs the scalar ring."""
    import contextlib

    import concourse.mybir as mybir
    import concourse.tile as tile
    from concourse import bacc

    nc = bacc.Bacc(None, target_bir_lowering=False, debug=False)

    f32 = mybir.dt.float32
    bf16 = mybir.dt.bfloat16
    odt = bf16 if out_bf16 else f32
    xT = nc.dram_tensor("xT", [D_IN, TOK_SH4], bf16, kind="ExternalInput")
    wTs = nc.dram_tensor("wTs", [D_IN, D_SH4], bf16, kind="ExternalInput")
    bias_col = nc.dram_tensor("bias_col", [P, DT4], f32, kind="ExternalInput")
    outT = nc.dram_tensor(
        "outT", [D_SH4, TOK_SH4], odt, kind="ExternalOutput")

    with tile.TileContext(nc) as tc:
        with tc.tile_pool(name="xr", bufs=KT) as xr_pool, \
             tc.tile_pool(name="wr", bufs=wr_bufs) as wr_pool, \
             tc.tile_pool(name="bias", bufs=1) as bias_pool, \
             tc.tile_pool(name="ob", bufs=ob_bufs) as ob_pool, \
             tc.tile_pool(name="ps", bufs=8, space="PSUM") as ps_pool:

            if repeats is not None:
                loop_cm = tc.For_i(
                    0, repeats, 1,
                    hint_engines=(
                        mybir.EngineType.PE, mybir.EngineType.DVE,
                        mybir.EngineType.Activation, mybir.EngineType.SP,
                        mybir.EngineType.Pool,
                    ),
                )
            else:
                loop_cm = contextlib.nullcontext()

            with loop_cm:
                bias_t = bias_pool.tile([P, DT4], f32, tag="bias")
                nc.sync.dma_start(bias_t[:], bias_col[:])
                x_r = [None] * KT

                def load_w(k, d):
                    wr = wr_pool.tile([P, P], bf16, tag="wr")
                    nc.sync.dma_start(
                        wr[:], wTs[k * P:(k + 1) * P, d * P:(d + 1) * P])
                    return wr

                def mm4(psums, wr, k, start, stop):
                    for t in range(TB4):
                        nc.tensor.matmul(
                            psums[t][:],
                            wr[:],
                            x_r[k][:, t * NF:(t + 1) * NF],
                            start=start,
                            stop=stop,
                        )

                def evict(psums, d):
                    for t in range(TB4):
                        ob = ob_pool.tile([P, NF], odt, tag="ob")
                        nc.vector.tensor_scalar_add(
                            ob[:], psums[t][:], bias_t[:, d:d + 1])
                        nc.scalar.dma_start(
                            outT[d * P:(d + 1) * P, t * NF:(t + 1) * NF],
                            ob[:])

                # paired d0/d1 phase: overlap the full x stream
                psA = [ps_pool.tile([P, NF], f32, tag="ps", name="ps")
                       for _ in range(TB4)]
                psB = [ps_pool.tile([P, NF], f32, tag="ps", name="ps")
                       for _ in range(TB4)]
                for k in range(KT):
                    # x striped in thirds over the 3 DGE rings (64B-
                    # aligned boundaries) so the 16 MB head stream can hit
                    # the full per-core HBM rate
                    xr = xr_pool.tile([P, TOK_SH4], bf16, tag="xr")
                    nc.scalar.dma_start(
                        xr[:, :672], xT[k * P:(k + 1) * P, :672])
                    nc.gpsimd.dma_start(
                        xr[:, 672:1360], xT[k * P:(k + 1) * P, 672:1360])
                    nc.sync.dma_start(
                        xr[:, 1360:], xT[k * P:(k + 1) * P, 1360:])
                    x_r[k] = xr
                    mm4(psA, load_w(k, 0), k, k == 0, k == KT - 1)
                    mm4(psB, load_w(k, 1), k, k == 0, k == KT - 1)
                evict(psA, 0)
                evict(psB, 1)

                for d in range(2, DT4):
                    psums = [ps_pool.tile([P, NF], f32, tag="ps", name="ps")
                             for _ in range(TB4)]
                    for k in range(KT):
                        mm4(psums, load_w(k, d), k, k == 0, k == KT - 1)
                    evict(psums, d)

    nc.compile()
    return nc


def _build_nc_wstat4c(repeats=None, wr_bufs=16, ob_bufs=8):
    """wstat4b with bf16 outputs: halves the out DMA (16->8 MB per core);
    the host upcasts to fp32 during assembly."""
    return _build_nc_wstat4b_impl(repeats, wr_bufs, ob_bufs, out_bf16=True)



def _dedup_ldweights(nc):
    """Remove consecutive duplicate InstLdweights (identical weights AP, no
    sync info): the PE keeps the stationary operand loaded across matmuls, so
    only the first load of each weight tile is needed.  Saves the per-matmul
    LDWEIGHTS issue/occupancy cost (~53-107ns each) on hardware; the Tile
    split pass emits one LDWEIGHTS per matmul even when lhsT is unchanged."""
    import concourse.mybir as mybir

    n_del = 0
    for b in nc.m.functions[0].blocks:
        prev_ap = None
        keep = []
        for ins in b.instructions:
            if isinstance(ins, mybir.InstLdweights):
                si = ins.sync_info
                empty = (si is None) or (
                    len(si.on_wait) == 0 and len(si.on_update) == 0)
                ap_r = repr(ins.ins[0])
                if empty and prev_ap == ap_r:
                    n_del += 1
                    continue
                prev_ap = ap_r
            elif isinstance(ins, mybir.InstMatmult):
                pass
            elif ins.engine == mybir.EngineType.PE:
                prev_ap = None
            keep.append(ins)
        b.instructions[:] = keep
    return n_del


def _build_nc_wstat4f(repeats=None, wr_bufs=3, ob_bufs=8, dedup_ldw=False,
                      out_bf16=False):
    """wstat4b with host-relayout W: each d-tile's 32 k-slabs are packed
    contiguously on host so the kernel issues ONE [128, 4096] W DMA per
    d-tile (8 KB segments) instead of 32x 32KB tile DMAs (256 B segments),
    cutting DMA descriptors and PE-side semaphore waits 32x."""
    import contextlib

    import concourse.mybir as mybir
    import concourse.tile as tile
    from concourse import bacc

    nc = bacc.Bacc(None, target_bir_lowering=False, debug=False)

    f32 = mybir.dt.float32
    bf16 = mybir.dt.bfloat16
    odt = bf16 if out_bf16 else f32
    WCOL = KT * P  # 4096 cols per packed d-tile block
    xT = nc.dram_tensor("xT", [D_IN, TOK_SH4], bf16, kind="ExternalInput")
    wT2 = nc.dram_tensor("wT2", [P, DT4 * WCOL], bf16, kind="ExternalInput")
    bias_col = nc.dram_tensor("bias_col", [P, DT4], f32, kind="ExternalInput")
    outT = nc.dram_tensor(
        "outT", [D_SH4, TOK_SH4], odt, kind="ExternalOutput")

    with tile.TileContext(nc) as tc:
        with tc.tile_pool(name="xr", bufs=KT) as xr_pool, \
             tc.tile_pool(name="wr", bufs=wr_bufs) as wr_pool, \
             tc.tile_pool(name="bias", bufs=1) as bias_pool, \
             tc.tile_pool(name="ob", bufs=ob_bufs) as ob_pool, \
             tc.tile_pool(name="ps", bufs=8, space="PSUM") as ps_pool:

            if repeats is not None:
                loop_cm = tc.For_i(
                    0, repeats, 1,
                    hint_engines=(
                        mybir.EngineType.PE, mybir.EngineType.DVE,
                        mybir.EngineType.Activation, mybir.EngineType.SP,
                        mybir.EngineType.Pool,
                    ),
                )
            else:
                loop_cm = contextlib.nullcontext()

            with loop_cm:
                bias_t = bias_pool.tile([P, DT4], f32, tag="bias")
                nc.sync.dma_start(bias_t[:], bias_col[:])
                x_r = [None] * KT

                def load_wd(d):
                    wr = wr_pool.tile([P, WCOL], bf16, tag="wr")
                    nc.sync.dma_start(
                        wr[:], wT2[:, d * WCOL:(d + 1) * WCOL])
                    return wr

                def mm4(psums, wr, k, start, stop):
                    for t in range(TB4):
                        nc.tensor.matmul(
                            psums[t][:],
                            wr[:, k * P:(k + 1) * P],
                            x_r[k][:, t * NF:(t + 1) * NF],
                            start=start,
                            stop=stop,
                        )

                def evict(psums, d):
                    for t in range(TB4):
                        ob = ob_pool.tile([P, NF], odt, tag="ob")
                        nc.vector.tensor_scalar_add(
                            ob[:], psums[t][:], bias_t[:, d:d + 1])
                        nc.scalar.dma_start(
                            outT[d * P:(d + 1) * P, t * NF:(t + 1) * NF],
                            ob[:])

                psA = [ps_pool.tile([P, NF], f32, tag="ps", name="ps")
                       for _ in range(TB4)]
                psB = [ps_pool.tile([P, NF], f32, tag="ps", name="ps")
                       for _ in range(TB4)]
                wr0, wr1 = load_wd(0), load_wd(1)
                for k in range(KT):
                    xr = xr_pool.tile([P, TOK_SH4], bf16, tag="xr")
                    nc.scalar.dma_start(
                        xr[:, :672], xT[k * P:(k + 1) * P, :672])
                    nc.gpsimd.dma_start(
                        xr[:, 672:1360], xT[k * P:(k + 1) * P, 672:1360])
                    nc.sync.dma_start(
                        xr[:, 1360:], xT[k * P:(k + 1) * P, 1360:])
                    x_r[k] = xr
                    mm4(psA, wr0, k, k == 0, k == KT - 1)
                    mm4(psB, wr1, k, k == 0, k == KT - 1)
                evict(psA, 0)
                evict(psB, 1)

                for d in range(2, DT4):
                    psums = [ps_pool.tile([P, NF], f32, tag="ps", name="ps")
                             for _ in range(TB4)]
                    wr = load_wd(d)
                    for k in range(KT):
                        mm4(psums, wr, k, k == 0, k == KT - 1)
                    evict(psums, d)

    if dedup_ldw:
        _dedup_ldweights(nc)
    nc.compile()
    return nc


def _build_nc_wstat4g(repeats=None, wr_bufs=3, ob_bufs=8):
    """wstat4f + LDWEIGHTS dedup: one weight load per 128x128 W tile instead
    of one per matmul (4x fewer)."""
    return _build_nc_wstat4f(repeats, wr_bufs, ob_bufs, dedup_ldw=True)


def _build_nc_wstat4i(repeats=None, wr_bufs=3, ob_bufs=8, dedup_ldw=False):
    """wstat4f with t-outer/k-inner steady-state loop: each PSUM bank
    receives 32 consecutive accumulating matmuls (bank switch every 32 MMs
    instead of every MM -- kills the K18 psum-cycling micro-idle), and each
    bank is evicted immediately after its k-sweep so the eviction of bank t
    overlaps the k-sweep of bank t+1.  Head phase (d0/d1 paired, k-outer to
    overlap the x stream) unchanged."""
    import contextlib

    import concourse.mybir as mybir
    import concourse.tile as tile
    from concourse import bacc

    nc = bacc.Bacc(None, target_bir_lowering=False, debug=False)

    f32 = mybir.dt.float32
    bf16 = mybir.dt.bfloat16
    WCOL = KT * P
    xT = nc.dram_tensor("xT", [D_IN, TOK_SH4], bf16, kind="ExternalInput")
    wT2 = nc.dram_tensor("wT2", [P, DT4 * WCOL], bf16, kind="ExternalInput")
    bias_col = nc.dram_tensor("bias_col", [P, DT4], f32, kind="ExternalInput")
    outT = nc.dram_tensor(
        "outT", [D_SH4, TOK_SH4], f32, kind="ExternalOutput")

    with tile.TileContext(nc) as tc:
        with tc.tile_pool(name="xr", bufs=KT) as xr_pool, \
             tc.tile_pool(name="wr", bufs=wr_bufs) as wr_pool, \
             tc.tile_pool(name="bias", bufs=1) as bias_pool, \
             tc.tile_pool(name="ob", bufs=ob_bufs) as ob_pool, \
             tc.tile_pool(name="ps", bufs=8, space="PSUM") as ps_pool:

            if repeats is not None:
                loop_cm = tc.For_i(
                    0, repeats, 1,
                    hint_engines=(
                        mybir.EngineType.PE, mybir.EngineType.DVE,
                        mybir.EngineType.Activation, mybir.EngineType.SP,
                        mybir.EngineType.Pool,
                    ),
                )
            else:
                loop_cm = contextlib.nullcontext()

            with loop_cm:
                bias_t = bias_pool.tile([P, DT4], f32, tag="bias")
                nc.sync.dma_start(bias_t[:], bias_col[:])
                x_r = [None] * KT

                def load_wd(d):
                    wr = wr_pool.tile([P, WCOL], bf16, tag="wr")
                    nc.sync.dma_start(
                        wr[:], wT2[:, d * WCOL:(d + 1) * WCOL])
                    return wr

                def mm4(psums, wr, k, start, stop):
                    for t in range(TB4):
                        nc.tensor.matmul(
                            psums[t][:],
                            wr[:, k * P:(k + 1) * P],
                            x_r[k][:, t * NF:(t + 1) * NF],
                            start=start,
                            stop=stop,
                        )

                def evict1(psum, d, t):
                    ob = ob_pool.tile([P, NF], f32, tag="ob")
                    nc.vector.tensor_scalar_add(
                        ob[:], psum[:], bias_t[:, d:d + 1])
                    nc.scalar.dma_start(
                        outT[d * P:(d + 1) * P, t * NF:(t + 1) * NF],
                        ob[:])

                # head: d0/d1 paired, k-outer so compute overlaps x stream
                psA = [ps_pool.tile([P, NF], f32, tag="ps", name="ps")
                       for _ in range(TB4)]
                psB = [ps_pool.tile([P, NF], f32, tag="ps", name="ps")
                       for _ in range(TB4)]
                wr0, wr1 = load_wd(0), load_wd(1)
                for k in range(KT):
                    xr = xr_pool.tile([P, TOK_SH4], bf16, tag="xr")
                    nc.scalar.dma_start(
                        xr[:, :672], xT[k * P:(k + 1) * P, :672])
                    nc.gpsimd.dma_start(
                        xr[:, 672:1360], xT[k * P:(k + 1) * P, 672:1360])
                    nc.sync.dma_start(
                        xr[:, 1360:], xT[k * P:(k + 1) * P, 1360:])
                    x_r[k] = xr
                    mm4(psA, wr0, k, k == 0, k == KT - 1)
                    mm4(psB, wr1, k, k == 0, k == KT - 1)
                for t in range(TB4):
                    evict1(psA[t], 0, t)
                for t in range(TB4):
                    evict1(psB[t], 1, t)

                # steady state: t-outer, k-inner, immediate eviction
                for d in range(2, DT4):
                    psums = [ps_pool.tile([P, NF], f32, tag="ps", name="ps")
                             for _ in range(TB4)]
                    wr = load_wd(d)
                    for t in range(TB4):
                        for k in range(KT):
                            nc.tensor.matmul(
                                psums[t][:],
                                wr[:, k * P:(k + 1) * P],
                                x_r[k][:, t * NF:(t + 1) * NF],
                                start=(k == 0),
                                stop=(k == KT - 1),
                            )
                        evict1(psums[t], d, t)

    if dedup_ldw:
        _dedup_ldweights(nc)
    nc.compile()
    return nc


def _build_nc_ex(repeats=None, no_w_dma=False, no_out_dma=False,
                 no_x_dma=False, no_evict=False, dedup_ldw=True):
    """Ablation builds of wstat4f for bottleneck decomposition (outputs are
    garbage when any flag is set -- timing only)."""
    import contextlib

    import concourse.mybir as mybir
    import concourse.tile as tile
    from concourse import bacc

    nc = bacc.Bacc(None, target_bir_lowering=False, debug=False)

    f32 = mybir.dt.float32
    bf16 = mybir.dt.bfloat16
    WCOL = KT * P
    xT = nc.dram_tensor("xT", [D_IN, TOK_SH4], bf16, kind="ExternalInput")
    wT2 = nc.dram_tensor("wT2", [P, DT4 * WCOL], bf16, kind="ExternalInput")
    bias_col = nc.dram_tensor("bias_col", [P, DT4], f32, kind="ExternalInput")
    outT = nc.dram_tensor(
        "outT", [D_SH4, TOK_SH4], f32, kind="ExternalOutput")

    with tile.TileContext(nc) as tc:
        with tc.tile_pool(name="xr", bufs=KT) as xr_pool, \
             tc.tile_pool(name="wr", bufs=3) as wr_pool, \
             tc.tile_pool(name="bias", bufs=1) as bias_pool, \
             tc.tile_pool(name="ob", bufs=8) as ob_pool, \
             tc.tile_pool(name="ps", bufs=8, space="PSUM") as ps_pool:

            if repeats is not None:
                loop_cm = tc.For_i(
                    0, repeats, 1,
                    hint_engines=(
                        mybir.EngineType.PE, mybir.EngineType.DVE,
                        mybir.EngineType.Activation, mybir.EngineType.SP,
                        mybir.EngineType.Pool,
                    ),
                )
            else:
                loop_cm = contextlib.nullcontext()

            with loop_cm:
                bias_t = bias_pool.tile([P, DT4], f32, tag="bias")
                nc.sync.dma_start(bias_t[:], bias_col[:])
                x_r = [None] * KT

                def load_wd(d):
                    wr = wr_pool.tile([P, WCOL], bf16, tag="wr")
                    nc.sync.dma_start(
                        wr[:], wT2[:, d * WCOL:(d + 1) * WCOL])
                    return wr

                def mm4(psums, wr, k, start, stop):
                    for t in range(TB4):
                        nc.tensor.matmul(
                            psums[t][:],
                            wr[:, k * P:(k + 1) * P],
                            x_r[k][:, t * NF:(t + 1) * NF],
                            start=start,
                            stop=stop,
                        )

                def evict(psums, d):
                    if no_evict:
                        return
                    for t in range(TB4):
                        ob = ob_pool.tile([P, NF], f32, tag="ob")
                        nc.vector.tensor_scalar_add(
                            ob[:], psums[t][:], bias_t[:, d:d + 1])
                        if not no_out_dma:
                            nc.scalar.dma_start(
                                outT[d * P:(d + 1) * P,
                                     t * NF:(t + 1) * NF],
                                ob[:])

                psA = [ps_pool.tile([P, NF], f32, tag="ps", name="ps")
                       for _ in range(TB4)]
                psB = [ps_pool.tile([P, NF], f32, tag="ps", name="ps")
                       for _ in range(TB4)]
                wr0, wr1 = load_wd(0), load_wd(1)
                for k in range(KT):
                    if no_x_dma and k > 0:
                        x_r[k] = x_r[0]
                    else:
                        xr = xr_pool.tile([P, TOK_SH4], bf16, tag="xr")
                        nc.scalar.dma_start(
                            xr[:, :672], xT[k * P:(k + 1) * P, :672])
                        nc.gpsimd.dma_start(
                            xr[:, 672:1360], xT[k * P:(k + 1) * P, 672:1360])
                        nc.sync.dma_start(
                            xr[:, 1360:], xT[k * P:(k + 1) * P, 1360:])
                        x_r[k] = xr
                    mm4(psA, wr0, k, k == 0, k == KT - 1)
                    mm4(psB, wr1, k, k == 0, k == KT - 1)
                evict(psA, 0)
                evict(psB, 1)

                for d in range(2, DT4):
                    psums = [ps_pool.tile([P, NF], f32, tag="ps", name="ps")
                             for _ in range(TB4)]
                    wr = psums and (wr0 if no_w_dma else load_wd(d))
                    for k in range(KT):
                        mm4(psums, wr, k, k == 0, k == KT - 1)
                    evict(psums, d)

    if dedup_ldw:
        _dedup_ldweights(nc)
    nc.compile()
    return nc


def _build_nc_wstat4h(repeats=None, wr_bufs=3, ob_bufs=8):
    """wstat4f + bf16 outputs (halves out-DMA traffic; host upcasts)."""
    return _build_nc_wstat4f(repeats, wr_bufs, ob_bufs, out_bf16=True)


def _build_nc_wstat4j(repeats=None, wr_bufs=3, ob_bufs=8):
    """wstat4i (t-outer/k-inner steady state, immediate per-bank eviction)
    + bf16 outputs + head W slabs d0/d1 split into 8-k chunks so the first
    LDWEIGHTS waits on 256 KB, not 1 MB."""
    import contextlib

    import concourse.mybir as mybir
    import concourse.tile as tile
    from concourse import bacc

    nc = bacc.Bacc(None, target_bir_lowering=False, debug=False)

    f32 = mybir.dt.float32
    bf16 = mybir.dt.bfloat16
    WCOL = KT * P
    xT = nc.dram_tensor("xT", [D_IN, TOK_SH4], bf16, kind="ExternalInput")
    wT2 = nc.dram_tensor("wT2", [P, DT4 * WCOL], bf16, kind="ExternalInput")
    bias_col = nc.dram_tensor("bias_col", [P, DT4], f32, kind="ExternalInput")
    outT = nc.dram_tensor(
        "outT", [D_SH4, TOK_SH4], bf16, kind="ExternalOutput")

    with tile.TileContext(nc) as tc:
        with tc.tile_pool(name="xr", bufs=KT) as xr_pool, \
             tc.tile_pool(name="wr", bufs=wr_bufs) as wr_pool, \
             tc.tile_pool(name="bias", bufs=1) as bias_pool, \
             tc.tile_pool(name="ob", bufs=ob_bufs) as ob_pool, \
             tc.tile_pool(name="ps", bufs=8, space="PSUM") as ps_pool:

            if repeats is not None:
                loop_cm = tc.For_i(
                    0, repeats, 1,
                    hint_engines=(
                        mybir.EngineType.PE, mybir.EngineType.DVE,
                        mybir.EngineType.Activation, mybir.EngineType.SP,
                        mybir.EngineType.Pool,
                    ),
                )
            else:
                loop_cm = contextlib.nullcontext()

            with loop_cm:
                bias_t = bias_pool.tile([P, DT4], f32, tag="bias")
                nc.sync.dma_start(bias_t[:], bias_col[:])
                x_r = [None] * KT

                def load_wd(d, chunks=1):
                    wr = wr_pool.tile([P, WCOL], bf16, tag="wr")
                    cw = WCOL // chunks
                    for c in range(chunks):
                        nc.sync.dma_start(
                            wr[:, c * cw:(c + 1) * cw],
                            wT2[:, d * WCOL + c * cw:d * WCOL + (c + 1) * cw])
                    return wr

                def mm4(psums, wr, k, start, stop):
                    for t in range(TB4):
                        nc.tensor.matmul(
                            psums[t][:],
                            wr[:, k * P:(k + 1) * P],
                            x_r[k][:, t * NF:(t + 1) * NF],
                            start=start,
                            stop=stop,
                        )

                def evict1(psum, d, t):
                    ob = ob_pool.tile([P, NF], bf16, tag="ob")
                    nc.vector.tensor_scalar_add(
                        ob[:], psum[:], bias_t[:, d:d + 1])
                    nc.scalar.dma_start(
                        outT[d * P:(d + 1) * P, t * NF:(t + 1) * NF],
                        ob[:])

                psA = [ps_pool.tile([P, NF], f32, tag="ps", name="ps")
                       for _ in range(TB4)]
                psB = [ps_pool.tile([P, NF], f32, tag="ps", name="ps")
                       for _ in range(TB4)]
                wr0, wr1 = load_wd(0, chunks=4), load_wd(1, chunks=2)
                for k in range(KT):
                    xr = xr_pool.tile([P, TOK_SH4], bf16, tag="xr")
                    nc.scalar.dma_start(
                        xr[:, :672], xT[k * P:(k + 1) * P, :672])
                    nc.gpsimd.dma_start(
                        xr[:, 672:1360], xT[k * P:(k + 1) * P, 672:1360])
                    nc.sync.dma_start(
                        xr[:, 1360:], xT[k * P:(k + 1) * P, 1360:])
                    x_r[k] = xr
                    mm4(psA, wr0, k, k == 0, k == KT - 1)
                    mm4(psB, wr1, k, k == 0, k == KT - 1)
                for t in range(TB4):
                    evict1(psA[t], 0, t)
                for t in range(TB4):
                    evict1(psB[t], 1, t)

                for d in range(2, DT4):
                    psums = [ps_pool.tile([P, NF], f32, tag="ps", name="ps")
                             for _ in range(TB4)]
                    wr = load_wd(d)
                    for t in range(TB4):
                        for k in range(KT):
                            nc.tensor.matmul(
                                psums[t][:],
                                wr[:, k * P:(k + 1) * P],
                                x_r[k][:, t * NF:(t + 1) * NF],
                                start=(k == 0),
                                stop=(k == KT - 1),
                            )
                        evict1(psums[t], d, t)

    nc.compile()
    return nc


def _build_nc_expi(repeats=None):
    """Pure-PE ablation with t-outer/k-inner ordering: same MM count as expe
    but PSUM bank switch every 32 MMs instead of every MM."""
    import contextlib

    import concourse.mybir as mybir
    import concourse.tile as tile
    from concourse import bacc

    nc = bacc.Bacc(None, target_bir_lowering=False, debug=False)

    f32 = mybir.dt.float32
    bf16 = mybir.dt.bfloat16
    WCOL = KT * P
    xT = nc.dram_tensor("xT", [D_IN, TOK_SH4], bf16, kind="ExternalInput")
    wT2 = nc.dram_tensor("wT2", [P, DT4 * WCOL], bf16, kind="ExternalInput")
    bias_col = nc.dram_tensor("bias_col", [P, DT4], f32, kind="ExternalInput")
    outT = nc.dram_tensor(
        "outT", [D_SH4, TOK_SH4], f32, kind="ExternalOutput")

    with tile.TileContext(nc) as tc:
        with tc.tile_pool(name="xr", bufs=1) as xr_pool, \
             tc.tile_pool(name="wr", bufs=1) as wr_pool, \
             tc.tile_pool(name="bias", bufs=1) as bias_pool, \
             tc.tile_pool(name="ps", bufs=8, space="PSUM") as ps_pool:

            if repeats is not None:
                loop_cm = tc.For_i(
                    0, repeats, 1,
                    hint_engines=(
                        mybir.EngineType.PE, mybir.EngineType.DVE,
                        mybir.EngineType.Activation, mybir.EngineType.SP,
                        mybir.EngineType.Pool,
                    ),
                )
            else:
                loop_cm = contextlib.nullcontext()

            with loop_cm:
                bias_t = bias_pool.tile([P, DT4], f32, tag="bias")
                nc.sync.dma_start(bias_t[:], bias_col[:])
                xr = xr_pool.tile([P, TOK_SH4], bf16, tag="xr")
                nc.scalar.dma_start(xr[:], xT[:P, :])
                wr = wr_pool.tile([P, WCOL], bf16, tag="wr")
                nc.sync.dma_start(wr[:], wT2[:, :WCOL])

                for d in range(DT4):
                    psums = [ps_pool.tile([P, NF], f32, tag="ps", name="ps")
                             for _ in range(TB4)]
                    for t in range(TB4):
                        for k in range(KT):
                            nc.tensor.matmul(
                                psums[t][:],
                                wr[:, k * P:(k + 1) * P],
                                xr[:, t * NF:(t + 1) * NF],
                                start=(k == 0),
                                stop=(k == KT - 1),
                            )

    nc.compile()
    return nc


def _build_nc_exn(repeats=None, NFE=256):
    """Pure-PE ablation with moving free dim NFE (same total moving columns
    as expi): per-MM overhead / clock discriminator."""
    import contextlib

    import concourse.mybir as mybir
    import concourse.tile as tile
    from concourse import bacc

    nc = bacc.Bacc(None, target_bir_lowering=False, debug=False)

    f32 = mybir.dt.float32
    bf16 = mybir.dt.bfloat16
    WCOL = KT * P
    NB = (TB4 * NF) // NFE  # moving blocks per d-tile
    xT = nc.dram_tensor("xT", [D_IN, TOK_SH4], bf16, kind="ExternalInput")
    wT2 = nc.dram_tensor("wT2", [P, DT4 * WCOL], bf16, kind="ExternalInput")
    bias_col = nc.dram_tensor("bias_col", [P, DT4], f32, kind="ExternalInput")
    outT = nc.dram_tensor(
        "outT", [D_SH4, TOK_SH4], f32, kind="ExternalOutput")

    with tile.TileContext(nc) as tc:
        with tc.tile_pool(name="xr", bufs=1) as xr_pool, \
             tc.tile_pool(name="wr", bufs=1) as wr_pool, \
             tc.tile_pool(name="bias", bufs=1) as bias_pool, \
             tc.tile_pool(name="ps", bufs=8, space="PSUM") as ps_pool:

            if repeats is not None:
                loop_cm = tc.For_i(
                    0, repeats, 1,
                    hint_engines=(
                        mybir.EngineType.PE, mybir.EngineType.DVE,
                        mybir.EngineType.Activation, mybir.EngineType.SP,
                        mybir.EngineType.Pool,
                    ),
                )
            else:
                loop_cm = contextlib.nullcontext()

            with loop_cm:
                bias_t = bias_pool.tile([P, DT4], f32, tag="bias")
                nc.sync.dma_start(bias_t[:], bias_col[:])
                xr = xr_pool.tile([P, TOK_SH4], bf16, tag="xr")
                nc.scalar.dma_start(xr[:], xT[:P, :])
                wr = wr_pool.tile([P, WCOL], bf16, tag="wr")
                nc.sync.dma_start(wr[:], wT2[:, :WCOL])

                for d in range(DT4):
                    psums = [ps_pool.tile([P, NFE], f32, tag="ps", name="ps")
                             for _ in range(4)]
                    for t in range(NB):
                        for k in range(KT):
                            nc.tensor.matmul(
                                psums[t % 4][:],
                                wr[:, k * P:(k + 1) * P],
                                xr[:, t * NFE:(t + 1) * NFE],
                                start=(k == 0),
                                stop=(k == KT - 1),
                            )

    nc.compile()
    return nc


def _build_nc_exn256(repeats=None):
    return _build_nc_exn(repeats, NFE=256)


def _build_nc_exn128(repeats=None):
    return _build_nc_exn(repeats, NFE=128)


def _build_nc_exw(repeats=None):
    return _build_nc_ex(repeats, no_w_dma=True)


def _build_nc_exwo(repeats=None):
    return _build_nc_ex(repeats, no_w_dma=True, no_out_dma=True)


def _build_nc_exwox(repeats=None):
    return _build_nc_ex(repeats, no_w_dma=True, no_out_dma=True,
                        no_x_dma=True)


def _build_nc_expe(repeats=None):
    return _build_nc_ex(repeats, no_w_dma=True, no_out_dma=True,
                        no_x_dma=True, no_evict=True)


def make_runner(nc, n_cores=N_CORES, replicated_inputs=()):
    """Build a reusable jitted SPMD callable for a compiled Bass module.

    Mirrors bass2jax.run_bass_via_pjrt's multi-core path, but returns the
    jitted function so repeated calls don't re-trace/re-compile.
    Inputs named in `replicated_inputs` use a replicated spec (pass the
    plain per-core array, no 8x concat)."""
    import jax
    import concourse.mybir as mybir
    from concourse import bass2jax
    from jax.experimental.shard_map import shard_map
    from jax.sharding import Mesh, PartitionSpec

    bass2jax.install_neuronx_cc_hook()

    partition_name = nc.partition_id_tensor.name if nc.partition_id_tensor else None
    in_names, out_names, out_avals, zero_outs = [], [], [], []
    for alloc in nc.m.functions[0].allocations:
        if not isinstance(alloc, mybir.MemoryLocationSet):
            continue
        name = alloc.memorylocations[0].name
        if alloc.kind == "ExternalInput":
            if name != partition_name:
                in_names.append(name)
        elif alloc.kind == "ExternalOutput":
            out_names.append(name)
            shape = tuple(alloc.tensor_shape)
            dtype = mybir.dt.np(alloc.dtype)
            out_avals.append(jax.core.ShapedArray(shape, dtype))
            zero_outs.append(np.zeros(shape, dtype))
    n_params = len(in_names)
    n_outs = len(out_avals)
    bind_in_names = list(in_names) + list(out_names)
    if partition_name is not None:
        bind_in_names.append(partition_name)

    def _body(*args):
        operands = list(args)
        if partition_name is not None:
            operands.append(bass2jax.partition_id_tensor())
        outs = bass2jax._bass_exec_p.bind(
            *operands,
            out_avals=tuple(out_avals),
            in_names=tuple(bind_in_names),
            out_names=tuple(out_names),
            lowering_input_output_aliases=(),
            sim_require_finite=True,
            sim_require_nnan=True,
            nc=nc,
        )
        return tuple(outs)

    devices = jax.devices()[:n_cores]
    mesh = Mesh(np.asarray(devices), ("core",))
    specs_map = {
        name: (PartitionSpec() if name in replicated_inputs
               else PartitionSpec("core"))
        for name in in_names
    }
    in_specs = tuple(specs_map[name] for name in in_names) + \
        (PartitionSpec("core"),) * n_outs
    out_specs = (PartitionSpec("core"),) * n_outs
    donate = tuple(range(n_params, n_params + n_outs))
    fn = jax.jit(
        shard_map(_body, mesh=mesh, in_specs=in_specs, out_specs=out_specs,
                  check_rep=False),
        donate_argnums=donate,
        keep_unused=True,
    )
    return {
        "fn": fn,
        "body": _body,
        "n_params": n_params,
        "in_names": in_names,
        "in_specs_map": specs_map,
        "out_names": out_names,
        "out_avals": out_avals,
        "zero_outs": zero_outs,
        "mesh": mesh,
        "n_cores": n_cores,
    }



DT8 = D_OUT // P   # 32 d-tiles per core (full D_OUT, 8-way token sharding)
TB8 = TOK_SH // NF  # 2 moving 512-token blocks per core


def _build_nc_wstat8(repeats=None, wr_bufs=4, ob_bufs=8):
    """8-way token sharding, full D_OUT per core, W-stationary with packed
    per-d-tile W slabs streamed (replicated W input).  x/core is only 8 MB so
    the head d0/d1 pair fully hides the x stream (4 MMs/k = ~1.1us vs
    ~0.8us/k of x DMA); W (32 MB) streams at ~58 GB/s over the whole kernel.
    Steady state is t-outer/k-inner with immediate per-bank eviction."""
    import contextlib

    import concourse.mybir as mybir
    import concourse.tile as tile
    from concourse import bacc

    nc = bacc.Bacc(None, target_bir_lowering=False, debug=False)

    f32 = mybir.dt.float32
    bf16 = mybir.dt.bfloat16
    WCOL = KT * P
    xT = nc.dram_tensor("xT", [D_IN, TOK_SH], bf16, kind="ExternalInput")
    wT8 = nc.dram_tensor(
        "wT8", [P, DT8 * WCOL], bf16, kind="ExternalInput")
    bias_col8 = nc.dram_tensor(
        "bias_col8", [P, DT8], f32, kind="ExternalInput")
    outT = nc.dram_tensor(
        "outT", [D_OUT, TOK_SH], f32, kind="ExternalOutput")

    with tile.TileContext(nc) as tc:
        with tc.tile_pool(name="xr", bufs=KT) as xr_pool, \
             tc.tile_pool(name="wr", bufs=wr_bufs) as wr_pool, \
             tc.tile_pool(name="bias", bufs=1) as bias_pool, \
             tc.tile_pool(name="ob", bufs=ob_bufs) as ob_pool, \
             tc.tile_pool(name="ps", bufs=8, space="PSUM") as ps_pool:

            if repeats is not None:
                loop_cm = tc.For_i(
                    0, repeats, 1,
                    hint_engines=(
                        mybir.EngineType.PE, mybir.EngineType.DVE,
                        mybir.EngineType.Activation, mybir.EngineType.SP,
                        mybir.EngineType.Pool,
                    ),
                )
            else:
                loop_cm = contextlib.nullcontext()

            with loop_cm:
                bias_t = bias_pool.tile([P, DT8], f32, tag="bias")
                nc.sync.dma_start(bias_t[:], bias_col8[:])
                x_r = [None] * KT

                def load_wd(d, chunks=1):
                    wr = wr_pool.tile([P, WCOL], bf16, tag="wr")
                    cw = WCOL // chunks
                    for c in range(chunks):
                        nc.sync.dma_start(
                            wr[:, c * cw:(c + 1) * cw],
                            wT8[:, d * WCOL + c * cw:d * WCOL + (c + 1) * cw])
                    return wr

                def evict1(psum, d, t):
                    ob = ob_pool.tile([P, NF], f32, tag="ob")
                    nc.vector.tensor_scalar_add(
                        ob[:], psum[:], bias_t[:, d:d + 1])
                    nc.scalar.dma_start(
                        outT[d * P:(d + 1) * P, t * NF:(t + 1) * NF],
                        ob[:])

                # head: d0/d1 paired, k-outer so compute hides the x stream
                psA = [ps_pool.tile([P, NF], f32, tag="ps", name="ps")
                       for _ in range(TB8)]
                psB = [ps_pool.tile([P, NF], f32, tag="ps", name="ps")
                       for _ in range(TB8)]
                wr0, wr1 = load_wd(0, chunks=4), load_wd(1, chunks=2)
                for k in range(KT):
                    xr = xr_pool.tile([P, TOK_SH], bf16, tag="xr")
                    nc.scalar.dma_start(
                        xr[:, :NF], xT[k * P:(k + 1) * P, :NF])
                    nc.gpsimd.dma_start(
                        xr[:, NF:], xT[k * P:(k + 1) * P, NF:])
                    x_r[k] = xr
                    for t in range(TB8):
                        nc.tensor.matmul(
                            psA[t][:], wr0[:, k * P:(k + 1) * P],
                            x_r[k][:, t * NF:(t + 1) * NF],
                            start=(k == 0), stop=(k == KT - 1))
                    for t in range(TB8):
                        nc.tensor.matmul(
                            psB[t][:], wr1[:, k * P:(k + 1) * P],
                            x_r[k][:, t * NF:(t + 1) * NF],
                            start=(k == 0), stop=(k == KT - 1))
                for t in range(TB8):
                    evict1(psA[t], 0, t)
                for t in range(TB8):
                    evict1(psB[t], 1, t)

                # steady state: t-outer/k-inner, immediate per-bank eviction
                for d in range(2, DT8):
                    psums = [ps_pool.tile([P, NF], f32, tag="ps", name="ps")
                             for _ in range(TB8)]
                    wr = load_wd(d)
                    for t in range(TB8):
                        for k in range(KT):
                            nc.tensor.matmul(
                                psums[t][:],
                                wr[:, k * P:(k + 1) * P],
                                x_r[k][:, t * NF:(t + 1) * NF],
                                start=(k == 0),
                                stop=(k == KT - 1),
                            )
                        evict1(psums[t], d, t)

    nc.compile()
    return nc


def _build_nc_wstat8c(repeats=None, wr_bufs=4, ob_bufs=8):
    """wstat8 with startup/tail shaving: d0 W slab chunked [2,6,24] k-tiles
    so the first LDWEIGHTS is gated on 64 KB (not 1 MB); bias DMA issued
    after that first chunk; the last d-tile tapers its t-sweep widths
    (512,512,512,256,128,128) so the final evict->out-DMA->sem chain rides
    on a 128-col tile."""
    import contextlib

    import concourse.mybir as mybir
    import concourse.tile as tile
    from concourse import bacc

    nc = bacc.Bacc(None, target_bir_lowering=False, debug=False)

    f32 = mybir.dt.float32
    bf16 = mybir.dt.bfloat16
    WCOL = KT * P
    xT = nc.dram_tensor("xT", [D_IN, TOK_SH], bf16, kind="ExternalInput")
    wT8 = nc.dram_tensor(
        "wT8", [P, DT8 * WCOL], bf16, kind="ExternalInput")
    bias_col8 = nc.dram_tensor(
        "bias_col8", [P, DT8], f32, kind="ExternalInput")
    outT = nc.dram_tensor(
        "outT", [D_OUT, TOK_SH], f32, kind="ExternalOutput")

    with tile.TileContext(nc) as tc:
        with tc.tile_pool(name="xr", bufs=KT) as xr_pool, \
             tc.tile_pool(name="wr", bufs=wr_bufs) as wr_pool, \
             tc.tile_pool(name="bias", bufs=1) as bias_pool, \
             tc.tile_pool(name="ob", bufs=ob_bufs) as ob_pool, \
             tc.tile_pool(name="ps", bufs=8, space="PSUM") as ps_pool:

            if repeats is not None:
                loop_cm = tc.For_i(
                    0, repeats, 1,
                    hint_engines=(
                        mybir.EngineType.PE, mybir.EngineType.DVE,
                        mybir.EngineType.Activation, mybir.EngineType.SP,
                        mybir.EngineType.Pool,
                    ),
                )
            else:
                loop_cm = contextlib.nullcontext()

            with loop_cm:
                x_r = [None] * KT

                def load_wd(d, kchunks=(KT,)):
                    wr = wr_pool.tile([P, WCOL], bf16, tag="wr")
                    k0 = 0
                    for nk in kchunks:
                        nc.sync.dma_start(
                            wr[:, k0 * P:(k0 + nk) * P],
                            wT8[:, d * WCOL + k0 * P:d * WCOL + (k0 + nk) * P])
                        k0 += nk
                    return wr

                wr0 = load_wd(0, kchunks=(2, 6, 24))
                bias_t = bias_pool.tile([P, DT8], f32, tag="bias")
                nc.sync.dma_start(bias_t[:], bias_col8[:])
                wr1 = load_wd(1, kchunks=(8, 24))

                def evict1(psum, d, c0, w):
                    ob = ob_pool.tile([P, w], f32, tag="ob")
                    nc.vector.tensor_scalar_add(
                        ob[:], psum[:], bias_t[:, d:d + 1])
                    nc.scalar.dma_start(
                        outT[d * P:(d + 1) * P, c0:c0 + w], ob[:])

                # head: d0/d1 paired, k-outer so compute hides the x stream
                psA = [ps_pool.tile([P, NF], f32, tag="ps", name="ps")
                       for _ in range(TB8)]
                psB = [ps_pool.tile([P, NF], f32, tag="ps", name="ps")
                       for _ in range(TB8)]
                for k in range(KT):
                    xr = xr_pool.tile([P, TOK_SH], bf16, tag="xr")
                    nc.scalar.dma_start(
                        xr[:, :NF], xT[k * P:(k + 1) * P, :NF])
                    nc.gpsimd.dma_start(
                        xr[:, NF:], xT[k * P:(k + 1) * P, NF:])
                    x_r[k] = xr
                    for t in range(TB8):
                        nc.tensor.matmul(
                            psA[t][:], wr0[:, k * P:(k + 1) * P],
                            x_r[k][:, t * NF:(t + 1) * NF],
                            start=(k == 0), stop=(k == KT - 1))
                    for t in range(TB8):
                        nc.tensor.matmul(
                            psB[t][:], wr1[:, k * P:(k + 1) * P],
                            x_r[k][:, t * NF:(t + 1) * NF],
                            start=(k == 0), stop=(k == KT - 1))
                for t in range(TB8):
                    evict1(psA[t], 0, t * NF, NF)
                for t in range(TB8):
                    evict1(psB[t], 1, t * NF, NF)

                def sweep(psum, wr, d, c0, w, last):
                    for k in range(KT):
                        nc.tensor.matmul(
                            psum[:],
                            wr[:, k * P:(k + 1) * P],
                            x_r[k][:, c0:c0 + w],
                            start=(k == 0),
                            stop=(k == KT - 1),
                        )
                    evict1(psum, d, c0, w)

                # steady state: t-outer/k-inner, immediate per-bank eviction
                for d in range(2, DT8 - 1):
                    psums = [ps_pool.tile([P, NF], f32, tag="ps", name="ps")
                             for _ in range(TB8)]
                    wr = load_wd(d)
                    for t in range(TB8):
                        sweep(psums[t], wr, d, t * NF, NF, False)

                # last d-tile: tapered sweep widths to shrink the tail chain
                d = DT8 - 1
                widths = (512, 256, 128, 128)  # sums to TOK_SH=1024
                wr = load_wd(d)
                c0 = 0
                for i, w in enumerate(widths):
                    ps = ps_pool.tile([P, w], f32, tag="ps", name="ps")
                    sweep(ps, wr, d, c0, w, i == len(widths) - 1)
                    c0 += w

    nc.compile()
    return nc


def _repl_for(layout):
    """Replicated-input names for a layout (inputs passed un-concatenated)."""
    if layout in ("xstat", "xstat2", "xbf16", "xbf16v2"):
        return ("wT", "biasb")
    if layout in ("wstat8", "wstat8c"):
        return ("wT8", "bias_col8")
    if layout.startswith(("wstat4", "ex")):
        return ()
    return ("wT", "bias_col")


LAYOUT = "wstat8c"  # see build_nc builders for all variants


def build_nc(layout=None, repeats=None):
    layout = layout or LAYOUT
    builders = {"xstat": _build_nc, "xstat2": _build_nc_v2,
                "xbf16": _build_nc_bf16, "wbf16": _build_nc_wstat_bf16, "xbf16v2": _build_nc_bf16v2,
                "wstat4": _build_nc_wstat4, "wstat4b": _build_nc_wstat4b, "wstat4c": _build_nc_wstat4c, "wstat4f": _build_nc_wstat4f,
                "wstat4g": _build_nc_wstat4g, "wstat4i": _build_nc_wstat4i,
                "wstat4h": _build_nc_wstat4h, "wstat4j": _build_nc_wstat4j,
                "exw": _build_nc_exw, "exwo": _build_nc_exwo,
                "exwox": _build_nc_exwox, "expe": _build_nc_expe, "expi": _build_nc_expi,
                "exn256": _build_nc_exn256, "exn128": _build_nc_exn128,
                "wstat8": _build_nc_wstat8, "wstat8c": _build_nc_wstat8c,
                "wstat": _build_nc_wstat, "wstat2": _build_nc_wstat2}
    return builders[layout](repeats=repeats)


def _get_runner(layout=None):
    layout = layout or LAYOUT
    key = f"runner_{layout}"
    if key not in _cache:
        _cache[key] = make_runner(
            build_nc(layout=layout), replicated_inputs=_repl_for(layout))
    return _cache[key]


def _circulant_expand(kernel):
    # W[p*b+i, q*b+j] = kernel[p, q, (i-j) % b]
    p, q, b = kernel.shape
    idx = (np.arange(b)[:, None] - np.arange(b)[None, :]) % b
    kc = kernel[:, :, idx]  # (p, q, b_i, b_j)
    return kc.transpose(0, 2, 1, 3).reshape(p * b, q * b)


def prep_inputs(x, base_weight, base_bias, c3a_kernel, layout=None):
    """Host-side prep: fold c3a into the weight; emit per-input concat arrays
    (axis 0 concat over cores, as shard_map expects)."""
    layout = layout or LAYOUT
    if layout.startswith("ex"):
        layout = "wstat4f"
    x = np.asarray(x, dtype=np.float32)
    base_weight = np.asarray(base_weight, dtype=np.float32)
    base_bias = np.asarray(base_bias, dtype=np.float32)
    c3a_kernel = np.asarray(c3a_kernel, dtype=np.float32)

    w_comb = base_weight + _circulant_expand(c3a_kernel) * (1.0 / D_IN)
    wT = np.ascontiguousarray(w_comb.T)                      # [D_IN, D_OUT]
    xT = np.ascontiguousarray(x.reshape(TOK, D_IN).T)        # [D_IN, TOK]

    if layout in ("xbf16", "wbf16", "xbf16v2") or \
            layout.startswith(("wstat4", "wstat8")):
        import ml_dtypes
        wT = wT.astype(ml_dtypes.bfloat16)
        xT = xT.astype(ml_dtypes.bfloat16)

    if layout.startswith("wstat8"):
        xT_cat = np.concatenate(
            [xT[:, c * TOK_SH:(c + 1) * TOK_SH] for c in range(N_CORES)],
            axis=0)
        w4 = wT.reshape(KT, P, DT8, P).transpose(1, 2, 0, 3)
        wT8 = np.ascontiguousarray(w4.reshape(P, DT8 * KT * P))
        bias_col8 = np.ascontiguousarray(base_bias.reshape(DT8, P).T)
        return {"xT": xT_cat, "wT8": wT8, "bias_col8": bias_col8}

    if layout.startswith("wstat4"):
        # core c: token quarter tq = c % 4, dout half dh = c // 4
        xT_cat = np.concatenate(
            [xT[:, (c % 4) * TOK_SH4:((c % 4) + 1) * TOK_SH4]
             for c in range(N_CORES)], axis=0)
        bias_cat = np.concatenate(
            [np.ascontiguousarray(
                base_bias[(c // 4) * D_SH4:((c // 4) + 1) * D_SH4]
                .reshape(DT4, P).T)
             for c in range(N_CORES)], axis=0)
        if layout in ("wstat4f", "wstat4g", "wstat4h", "wstat4i", "wstat4j"):
            # pack each d-tile's 32 k-slabs contiguously: wT2[p, (d*KT+k)*P+c]
            # = wT[k*P+p, dh*D_SH4 + d*P+c]
            blocks = []
            for c in range(N_CORES):
                ws = wT[:, (c // 4) * D_SH4:((c // 4) + 1) * D_SH4]
                w4 = ws.reshape(KT, P, DT4, P).transpose(1, 2, 0, 3)
                blocks.append(np.ascontiguousarray(
                    w4.reshape(P, DT4 * KT * P)))
            return {"xT": xT_cat, "wT2": np.concatenate(blocks, axis=0),
                    "bias_col": bias_cat}
        wTs_cat = np.concatenate(
            [wT[:, (c // 4) * D_SH4:((c // 4) + 1) * D_SH4]
             for c in range(N_CORES)], axis=0)
        return {"xT": xT_cat, "wTs": wTs_cat, "bias_col": bias_cat}

    # per-core shards, concatenated along axis 0 (shard_map splits axis 0)
    xT_cat = np.concatenate(
        [xT[:, c * TOK_SH:(c + 1) * TOK_SH] for c in range(N_CORES)], axis=0)
    if layout in ("xstat", "xbf16", "xbf16v2"):
        biasb = np.ascontiguousarray(
            np.broadcast_to(base_bias, (P, D_OUT)).astype(np.float32))
        return {"xT": xT_cat, "wT": wT, "biasb": biasb}
    else:
        bias_col = np.ascontiguousarray(base_bias.reshape(D_OUT // P, P).T)
        return {"xT": xT_cat, "wT": wT, "bias_col": bias_col}


def assemble_output(out_global, layout=None):
    """out_global: the concat-over-cores output array -> full (B,S,D_OUT)."""
    layout = layout or LAYOUT
    if layout.startswith("ex"):
        layout = "wstat4f"
    if layout in ("xstat", "xstat2", "xbf16", "xbf16v2"):
        # (N_CORES*TOK_SH, D_OUT), token-sharded in order
        return np.asarray(out_global).reshape(B, S, D_OUT)
    elif layout.startswith("wstat8"):
        # (N_CORES*D_OUT, TOK_SH): core c holds outT[:, c*1024:(c+1)*1024]
        a = np.asarray(out_global).reshape(N_CORES, D_OUT, TOK_SH)
        full = a.transpose(1, 0, 2).reshape(D_OUT, TOK)
        return np.ascontiguousarray(full.T).reshape(B, S, D_OUT)
    elif layout.startswith("wstat4"):
        # (8*D_SH4, TOK_SH4): core c holds outT[dh*2048.., tq*2048..]
        a = np.asarray(out_global).reshape(N_CORES, D_SH4, TOK_SH4)
        full = np.empty((D_OUT, TOK), dtype=np.float32)
        for c in range(N_CORES):
            dh, tq = c // 4, c % 4
            full[dh * D_SH4:(dh + 1) * D_SH4,
                 tq * TOK_SH4:(tq + 1) * TOK_SH4] = a[c]
        return np.ascontiguousarray(full.T).reshape(B, S, D_OUT)
    else:
        # (N_CORES*D_OUT, TOK_SH) -> [c, d, t] -> full [d, c*t]
        a = np.asarray(out_global).reshape(N_CORES, D_OUT, TOK_SH)
        full = a.transpose(1, 0, 2).reshape(D_OUT, TOK)
        return np.ascontiguousarray(full.T).reshape(B, S, D_OUT)


def kernel(x, base_weight, base_bias, c3a_kernel, **_):
    runner = _get_runner()
    cat = prep_inputs(x, base_weight, base_bias, c3a_kernel)
    ins = [cat[name] for name in runner["in_names"]]
    zeros = [
        np.zeros((N_CORES * z.shape[0], *z.shape[1:]), z.dtype)
        for z in runner["zero_outs"]
    ]
    out_arrs = runner["fn"](*ins, *zeros)
    return assemble_output(out_arrs[0])

